# revision 1
# baseline (speedup 1.0000x reference)
"""Collective variant: K/V projection split across core pairs + pair AllGather.

Core c = (batch c//2, stripe h = c%2). Each core projects K^T and V only for
its own key half (s in [h*1024, (h+1)*1024)), then the pair exchanges halves
via two AllGathers (one per 512-key own-block) so attention can start as soon
as the first halves have been gathered.

Gathered DRAM layout (per 512-key global block b, r = b//2 = producing rank,
sub = b%2 selects which of the two collectives):
  cc = ccA if b%2==0 else ccB; base = r*2048
  KT tile k:  cc[base + k*128 : +128, :]                       [128, 512]
  V tile st:  cc[base + 1024 + st*256 : +256, :] as [128,1024] (row-pair fold)
"""

import numpy as np

B, S, E, KD = 4, 2048, 1024, 1024
NCORES = 8
P = 128
ET = E // P
KT = KD // P
NQT = 8
NBLK = 4
NEG = -30000.0
SCALE = 1.0 / float(np.sqrt(KD))

_prog_cache = {}


def _n_blocks(t):
    return (t + 2) // 2


def _build_body(ctx, tc, ap):
    from concourse import mybir
    from concourse.masks import make_identity

    nc = tc.nc
    f32 = mybir.dt.float32
    f32r = mybir.dt.float32r
    Exp = mybir.ActivationFunctionType.Exp
    X = mybir.AxisListType.X

    xTq_t = ap["xTq"].rearrange("(t p) q -> t p q", p=P)    # [8, 128, 1024]
    xTp_t = ap["xTp"].rearrange("(t p) s -> t p s", p=P)    # [8, 128, 1024]
    wqT_t = ap["wqT"].rearrange("(t p) k -> t p k", p=P)
    wkT_t = ap["wkT"].rearrange("(t p) k -> t p k", p=P)
    wvT_t = ap["wvT"].rearrange("(t p) f -> t p f", p=P)
    out_t = ap["out"].rearrange("(t p) f -> t p f", p=P)

    # ---- persistent tiles
    qt_pool = ctx.enter_context(tc.tile_pool(name="qt", bufs=1))
    QT = [qt_pool.tile([P, 1024], f32r, name=f"qt{k}", tag=f"qt{k}") for k in range(KT)]
    acc_pool = ctx.enter_context(tc.tile_pool(name="acc", bufs=1))
    OACC = [acc_pool.tile([P, E], f32, name=f"oacc{t}", tag=f"oacc{t}") for t in range(NQT)]
    RS = [acc_pool.tile([P, NBLK], f32, name=f"rs{t}", tag=f"rs{t}") for t in range(NQT)]
    const_pool = ctx.enter_context(tc.tile_pool(name="const", bufs=1))
    fin_pool = ctx.enter_context(tc.tile_pool(name="fin", bufs=4))

    # ---- DRAM tiles for the pair exchange
    dram = ctx.enter_context(tc.tile_pool(name="dram", bufs=1, space="DRAM"))
    ccin = [dram.tile([2048, 512], f32r, name=f"ccin{i}", tag=f"ccin{i}") for i in range(2)]
    ccout = [dram.tile([4096, 512], f32r, name=f"ccout{i}", tag=f"ccout{i}") for i in range(2)]

    # ---- PSUM: pp (projection evict) lives only through the projection
    # phases; its banks are then handed to the attention pools (vp bufs=2).
    pp_ctx = tc.tile_pool(name="pp", bufs=4, space="PSUM")
    pp = pp_ctx.__enter__()

    # ---- Phase A: own-half K/V projection + pair exchange.
    # Emitted FIRST so the K/V inputs arrive first and the collectives launch
    # as early as possible; the QT projection then runs underneath the
    # collective latency instead of in front of it.
    with tc.tile_pool(name="wkp", bufs=1) as wk_pool, \
         tc.tile_pool(name="wvp", bufs=1) as wv_pool, \
         tc.tile_pool(name="xpp", bufs=1) as xp_pool, \
         tc.tile_pool(name="stg", bufs=2) as stg_pool:
        wk = [wk_pool.tile([P, KD], f32r, name=f"wk{e}", tag=f"wk{e}") for e in range(ET)]
        xp = [xp_pool.tile([P, 1024], f32r, name=f"xp{e}", tag=f"xp{e}") for e in range(ET)]
        wv = [wv_pool.tile([P, E], f32r, name=f"wv{e}", tag=f"wv{e}") for e in range(ET)]
        # arrival order tuned to keep PE continuously fed:
        # [xp0+wk] -> xp1 -> wv -> (wq, xq emitted in phase B)
        for e in range(ET):
            nc.sync.dma_start(out=xp[e], in_=xTp_t[e])
            nc.sync.dma_start(out=wk[e], in_=wkT_t[e])
        for e in range(ET):
            nc.sync.dma_start(out=wv[e], in_=wvT_t[e])

        for ob in range(2):
            for k in range(KT):
                ps = pp.tile([P, 512], f32, name="ps_kt", tag="pp")
                for e in range(ET):
                    nc.tensor.matmul(ps, wk[e][:, k * P:(k + 1) * P],
                                     xp[e][:, ob * 512:(ob + 1) * 512],
                                     start=(e == 0), stop=(e == ET - 1))
                ko = stg_pool.tile([P, 512], f32r, name="ko", tag="ko", bufs=3)
                nc.vector.tensor_copy(ko, ps)
                nc.gpsimd.dma_start(out=ccin[ob][k * P:(k + 1) * P, :], in_=ko)
            # V_own[ob]: [512, 1024] -> rows 1024: as [1024, 512] row-pair fold
            for st in range(4):
                vo = stg_pool.tile([P, E], f32r, name="vo", tag="vo", bufs=3)
                for fb in range(2):
                    ps = pp.tile([P, 512], f32, name="ps_v", tag="pp")
                    for e in range(ET):
                        nc.tensor.matmul(
                            ps, xp[e][:, ob * 512 + st * P: ob * 512 + (st + 1) * P],
                            wv[e][:, fb * 512:(fb + 1) * 512],
                            start=(e == 0), stop=(e == ET - 1))
                    if fb == 0:
                        nc.scalar.copy(vo[:, fb * 512:(fb + 1) * 512], ps)
                    else:
                        nc.vector.tensor_copy(vo[:, fb * 512:(fb + 1) * 512], ps)
                vdst = ccin[ob][1024 + st * 256: 1024 + (st + 1) * 256, :]
                nc.gpsimd.dma_start(
                    out=vdst.rearrange("(s a) c -> s (a c)", a=2), in_=vo)
            nc.gpsimd.collective_compute(
                "AllGather", mybir.AluOpType.bypass,
                replica_groups=[[0, 1], [2, 3], [4, 5], [6, 7]],
                ins=[ccin[ob].opt()], outs=[ccout[ob].opt()],
            )

    # ---- Phase B: QT[k, q] projection (runs while the collectives fly)
    with tc.tile_pool(name="wqp", bufs=1) as wq_pool, \
         tc.tile_pool(name="xqp", bufs=1) as xq_pool:
        wq = [wq_pool.tile([P, KD], f32r, name=f"wq{e}", tag=f"wq{e}") for e in range(ET)]
        xq = [xq_pool.tile([P, 1024], f32r, name=f"xq{e}", tag=f"xq{e}") for e in range(ET)]
        for e in range(ET):
            nc.sync.dma_start(out=wq[e], in_=wqT_t[e])
            nc.sync.dma_start(out=xq[e], in_=xTq_t[e])
        for qb in range(2):
            for k in range(KT):
                ps = pp.tile([P, 512], f32, name="ps_qt", tag="pp")
                for e in range(ET):
                    nc.tensor.matmul(
                        ps, wq[e][:, k * P:(k + 1) * P],
                        xq[e][:, qb * 512:(qb + 1) * 512],
                        start=(e == 0), stop=(e == ET - 1))
                if k % 2 == 0:
                    nc.vector.tensor_copy(QT[k][:, qb * 512:(qb + 1) * 512], ps)
                else:
                    nc.scalar.copy(QT[k][:, qb * 512:(qb + 1) * 512], ps)

    # ---- Phase C: attention over global blocks
    pp_ctx.__exit__(None, None, None)
    cm = const_pool.tile([P, 256], f32, name="cm")
    nc.sync.dma_start(out=cm, in_=ap["cmask"])
    ident_f32 = const_pool.tile([P, P], f32, name="ident_f32")
    make_identity(nc, ident_f32)
    ident = const_pool.tile([P, P], f32r, name="ident")
    nc.vector.tensor_copy(ident, ident_f32)
    sp = ctx.enter_context(tc.tile_pool(name="sp", bufs=2, space="PSUM"))
    tp = ctx.enter_context(tc.tile_pool(name="tp", bufs=2, space="PSUM"))
    vp = ctx.enter_context(tc.tile_pool(name="vp", bufs=2, space="PSUM"))
    kt_pool = ctx.enter_context(tc.tile_pool(name="ktp", bufs=2))
    vb_pool = ctx.enter_context(tc.tile_pool(name="vbp", bufs=2))
    p_pool = ctx.enter_context(tc.tile_pool(name="ppb", bufs=4))
    pt_pool = ctx.enter_context(tc.tile_pool(name="ptp", bufs=6))

    ORDER = (0, 2, 1, 3)  # blocks 0,2 come from CC1 — start before CC2 lands
    last_visit = {t: [b for b in ORDER if t >= 2 * b][-1] for t in range(NQT)}

    def emit_pv(pend):
        # deferred transpose/copy/PV/accumulate for one (blk, t) work item;
        # runs one position behind the scores stream so the PE->DVE->PE
        # transpose-copy chain and exp latency hide behind matmul work.
        pb, w, blk, t, vbt = pend
        nst = w // P
        tpss = []
        for st in range(nst):
            tps = tp.tile([P, P], f32r, name="tps", tag="tp")
            nc.tensor.transpose(tps, pb[:, st * P:(st + 1) * P], ident)
            tpss.append(tps)
            if st > 0:
                pts = pt_pool.tile([P, P], f32r, name="pts", tag=f"pt{st-1}")
                nc.vector.tensor_copy(pts, tpss[st - 1])
                tpss[st - 1] = pts
        pts = pt_pool.tile([P, P], f32r, name="pts", tag=f"pt{nst-1}")
        nc.vector.tensor_copy(pts, tpss[nst - 1])
        tpss[nst - 1] = pts
        vps = [vp.tile([P, 512], f32, name=f"vps{fb}", tag=f"vp{fb}") for fb in range(2)]
        for st in range(nst):
            for fb in range(2):
                nc.tensor.matmul(vps[fb], tpss[st],
                                 vbt[st][:, fb * 512:(fb + 1) * 512],
                                 start=(st == 0), stop=(st == nst - 1))
        for fb in range(2):
            dst = OACC[t][:, fb * 512:(fb + 1) * 512]
            if blk == 0:
                nc.vector.tensor_copy(dst, vps[fb])
            else:
                nc.vector.tensor_add(dst, dst, vps[fb])
        if blk == last_visit[t]:
            nb = _n_blocks(t)
            rsum = fin_pool.tile([P, 1], f32, name="rsum", tag="rsum")
            nc.vector.reduce_sum(rsum, RS[t][:, :nb], axis=X)
            rinv = fin_pool.tile([P, 1], f32, name="rinv", tag="rinv")
            nc.vector.reciprocal(rinv, rsum)
            nc.scalar.activation(OACC[t], OACC[t],
                                 mybir.ActivationFunctionType.Copy, scale=rinv)
            nc.sync.dma_start(out=out_t[t], in_=OACC[t])

    pending = None  # pipeline carries across block boundaries (vb bufs=2)
    for blk in ORDER:
        r, sub = blk // 2, blk % 2
        cc = ccout[sub]
        base = r * 2048
        ktb = [kt_pool.tile([P, 512], f32r, name=f"ktb{k}", tag=f"ktb{k}") for k in range(KT)]
        for k in range(KT):
            nc.sync.dma_start(out=ktb[k], in_=cc[base + k * P: base + (k + 1) * P, :])
        vbt = [vb_pool.tile([P, E], f32r, name=f"vb{st}", tag=f"vb{st}") for st in range(4)]
        for st in range(4):
            vsrc = cc[base + 1024 + st * 256: base + 1024 + (st + 1) * 256, :]
            nc.sync.dma_start(out=vbt[st], in_=vsrc.rearrange("(s a) c -> s (a c)", a=2))

        for t in range(2 * blk, NQT):
            w = min(512, 256 * (t + 1) - 512 * blk)
            is_diag = (blk == _n_blocks(t) - 1)
            sps = sp.tile([P, 512], f32, name="sps", tag="sp")
            for k in range(KT):
                nc.tensor.matmul(sps[:, :w], QT[k][:, t * P:(t + 1) * P],
                                 ktb[k][:, :w], start=(k == 0), stop=(k == KT - 1))
            if is_diag:
                nc.vector.tensor_add(sps[:, w - 256:w], sps[:, w - 256:w], cm)
            pb = p_pool.tile([P, 512], f32r, name="pb", tag="pb")
            nc.scalar.activation(pb[:, :w], sps[:, :w], Exp, scale=SCALE,
                                 accum_out=RS[t][:, blk:blk + 1])
            if pending is not None:
                emit_pv(pending)
            pending = (pb, w, blk, t, vbt)
    emit_pv(pending)


def build_program():
    if "nc" in _prog_cache:
        return _prog_cache["nc"]
    from contextlib import ExitStack
    from concourse import bacc, mybir
    import concourse.tile as tile

    nc = bacc.Bacc("TRN2", target_bir_lowering=False, debug=False,
                   num_devices=NCORES)
    f32 = mybir.dt.float32
    f32r = mybir.dt.float32r
    ap = {
        "xTq": nc.dram_tensor("xTq", [E, 1024], f32r, kind="ExternalInput").ap(),
        "xTp": nc.dram_tensor("xTp", [E, 1024], f32r, kind="ExternalInput").ap(),
        "wqT": nc.dram_tensor("wqT", [E, KD], f32r, kind="ExternalInput").ap(),
        "wkT": nc.dram_tensor("wkT", [E, KD], f32r, kind="ExternalInput").ap(),
        "wvT": nc.dram_tensor("wvT", [E, E], f32r, kind="ExternalInput").ap(),
        "cmask": nc.dram_tensor("cmask", [P, 256], f32, kind="ExternalInput").ap(),
        "out": nc.dram_tensor("out", [1024, E], f32, kind="ExternalOutput").ap(),
    }
    with tile.TileContext(nc) as tc:
        with ExitStack() as ctx:
            _build_body(ctx, tc, ap)
    nc.compile()
    _prog_cache["nc"] = nc
    return nc


def make_in_maps(x, W_q, W_k, W_v):
    x = np.ascontiguousarray(np.asarray(x, np.float32))
    wqT = np.ascontiguousarray(np.asarray(W_q, np.float32).T)
    wkT = np.ascontiguousarray(np.asarray(W_k, np.float32).T)
    wvT = np.ascontiguousarray(np.asarray(W_v, np.float32).T)
    i = np.arange(P)[:, None]
    j = np.arange(256)[None, :]
    cmasks = [np.where(j <= i + 128, 0.0, NEG).astype(np.float32),
              np.where(j <= i, 0.0, NEG).astype(np.float32)]
    in_maps = []
    for c in range(NCORES):
        b, h = c // 2, c % 2
        xT = np.ascontiguousarray(x[b].T)
        qtiles = [2 * t + (1 - h) for t in range(NQT)]
        qcols = np.concatenate([np.arange(g * P, (g + 1) * P) for g in qtiles])
        xTq = np.ascontiguousarray(xT[:, qcols])
        xTp = np.ascontiguousarray(xT[:, h * 1024:(h + 1) * 1024])
        in_maps.append({
            "xTq": xTq, "xTp": xTp, "wqT": wqT, "wkT": wkT, "wvT": wvT,
            "cmask": cmasks[h],
        })
    return in_maps


def assemble(results):
    out = np.zeros((B, S, E), np.float32)
    for c in range(NCORES):
        b, h = c // 2, c % 2
        co = results[c]["out"]
        for t in range(NQT):
            g = 2 * t + (1 - h)
            out[b, g * P:(g + 1) * P, :] = co[t * P:(t + 1) * P]
    return out


def kernel(x, W_q, W_k, W_v):
    from concourse.bass_utils import run_bass_kernel_spmd
    nc = build_program()
    in_maps = make_in_maps(x, W_q, W_k, W_v)
    res = run_bass_kernel_spmd(nc, in_maps, core_ids=list(range(NCORES)))
    return assemble(res.results)



# revision 7
# speedup vs baseline: 2.4693x; 2.4693x over previous
"""Collective-free causal attention: 8 cores = 4 batches x 2 q-stripe sets.

Core c = (batch b = c//2, stripe set h = c%2) owns the 8 query stripes
g = 2t + (1-h), t in 0..7, of batch b.  Each core projects the FULL K^T and V
for its batch locally (duplicated within the pair) instead of exchanging
halves over a collective — the cost-model charges intra-pair AllGathers like
inter-chip transfers (15us + size/40GBps, serialized), which dominated the
previous version; the duplicate K/V projection is ~27us of extra PE work.

All matmul inputs are bf16 (host-converted); accumulation stays fp32 in PSUM,
softmax row-sums and the output accumulator stay fp32.

Attention loop (per key block blk of 512, stripes t >= 2*blk):
  scores[q,s] in PSUM <- sum_k QT[k][:,t]^T KT[k][:,blk]   (bf16 matmuls)
  diag block: add causal mask; exp via ACT with fp32 row-sum accumulate
  P^T via PE transpose -> bf16; PV accumulated in PSUM over key subtiles
  OACC[t] += PV; on last block: scale by 1/rowsum and DMA out.
"""

import numpy as np

B, S, E, KD = 4, 2048, 1024, 1024
NCORES = 8
P = 128
ET = E // P          # 8 e-tiles of the contraction dim
KT = KD // P         # 8 k-tiles of Q^T/K^T partition dim
NQT = 8              # 8 q stripes of 128 per core
NBLK = 4             # 4 key blocks of 512
NST = S // P         # 16 key subtiles of 128 (V tiles)
NEG = -30000.0
SCALE = 1.0 / float(np.sqrt(KD))

_prog_cache = {}


def _n_blocks(t):
    return (t + 2) // 2


def _build_body(ctx, tc, ap):
    from concourse import mybir
    from concourse.masks import make_identity

    nc = tc.nc
    f32 = mybir.dt.float32
    bf16 = mybir.dt.bfloat16
    Exp = mybir.ActivationFunctionType.Exp
    X = mybir.AxisListType.X

    xT_t = ap["xT"].rearrange("(t p) s -> t p s", p=P)     # [8, 128, 2048]
    xTq_t = ap["xTq"].rearrange("(t p) q -> t p q", p=P)   # [8, 128, 1024]
    wqT_t = ap["wqT"].rearrange("(t p) k -> t p k", p=P)
    wkT_t = ap["wkT"].rearrange("(t p) k -> t p k", p=P)
    wvT_t = ap["wvT"].rearrange("(t p) f -> t p f", p=P)
    out_t = ap["out"].rearrange("(t p) f -> t p f", p=P)

    # ---- persistent tiles
    qt_pool = ctx.enter_context(tc.tile_pool(name="qt", bufs=1))
    QT = [qt_pool.tile([P, 1024], bf16, name=f"qt{k}", tag=f"qt{k}") for k in range(KT)]
    kt_pool = ctx.enter_context(tc.tile_pool(name="ktp", bufs=1))
    KTT = [kt_pool.tile([P, S], bf16, name=f"ktt{k}", tag=f"ktt{k}") for k in range(KT)]
    vv_pool = ctx.enter_context(tc.tile_pool(name="vvp", bufs=1))
    VV = [vv_pool.tile([P, E], bf16, name=f"vv{s}", tag=f"vv{s}") for s in range(NST)]
    rs_pool = ctx.enter_context(tc.tile_pool(name="rsp", bufs=1))
    RS = [rs_pool.tile([P, NBLK], f32, name=f"rs{t}", tag=f"rs{t}") for t in range(NQT)]
    const_pool = ctx.enter_context(tc.tile_pool(name="const", bufs=1))
    fin_pool = ctx.enter_context(tc.tile_pool(name="fin", bufs=4))

    cm = const_pool.tile([P, 256], f32, name="cm")
    nc.sync.dma_start(out=cm, in_=ap["cmask"])
    ident_f32 = const_pool.tile([P, P], f32, name="ident_f32")
    make_identity(nc, ident_f32)
    ident = const_pool.tile([P, P], bf16, name="ident")
    nc.vector.tensor_copy(ident, ident_f32)

    # PSUM pool for the projection phase; banks are handed to the attention
    # pools afterwards.
    pp_ctx = tc.tile_pool(name="pp", bufs=4, space="PSUM")
    pp = pp_ctx.__enter__()

    # GPSIMD cannot access PSUM, so evictions alternate DVE/ACT.
    evict_ops = [lambda d, s: nc.vector.tensor_copy(d, s),
                 lambda d, s: nc.scalar.copy(d, s)]
    evict_i = 0

    def evict(dst, src):
        nonlocal evict_i
        evict_ops[evict_i % 2](dst, src)
        evict_i += 1

    # ---- projection phase (scoped input pools)
    with tc.tile_pool(name="xtp", bufs=1) as xt_pool, \
         tc.tile_pool(name="xqp", bufs=1) as xq_pool, \
         tc.tile_pool(name="wqp", bufs=1) as wq_pool, \
         tc.tile_pool(name="wkp", bufs=1) as wk_pool, \
         tc.tile_pool(name="wvp", bufs=1) as wv_pool:
        xt = [xt_pool.tile([P, S], bf16, name=f"xt{e}", tag=f"xt{e}") for e in range(ET)]
        xq = [xq_pool.tile([P, 1024], bf16, name=f"xq{e}", tag=f"xq{e}") for e in range(ET)]
        wq = [wq_pool.tile([P, KD], bf16, name=f"wq{e}", tag=f"wq{e}") for e in range(ET)]
        wk = [wk_pool.tile([P, KD], bf16, name=f"wk{e}", tag=f"wk{e}") for e in range(ET)]
        wv = [wv_pool.tile([P, E], bf16, name=f"wv{e}", tag=f"wv{e}") for e in range(ET)]
        for e in range(ET):
            nc.sync.dma_start(out=xt[e], in_=xT_t[e])
            nc.sync.dma_start(out=wk[e], in_=wkT_t[e])
        for e in range(ET):
            nc.sync.dma_start(out=wv[e], in_=wvT_t[e])
        for e in range(ET):
            nc.sync.dma_start(out=xq[e], in_=xTq_t[e])
            nc.sync.dma_start(out=wq[e], in_=wqT_t[e])

        # K^T[k] = W_k^T[:,k]^T x^T : [128 kdim, 2048 keys]
        for k in range(KT):
            for sb in range(NBLK):
                ps = pp.tile([P, 512], f32, name="ps_k", tag="pp")
                for e in range(ET):
                    nc.tensor.matmul(ps, wk[e][:, k * P:(k + 1) * P],
                                     xt[e][:, sb * 512:(sb + 1) * 512],
                                     start=(e == 0), stop=(e == ET - 1))
                evict(KTT[k][:, sb * 512:(sb + 1) * 512], ps)
        # V[st] = x[st rows] W_v^T : [128 keys, 1024 features]
        for st in range(NST):
            for fb in range(2):
                ps = pp.tile([P, 512], f32, name="ps_v", tag="pp")
                for e in range(ET):
                    nc.tensor.matmul(ps, xt[e][:, st * P:(st + 1) * P],
                                     wv[e][:, fb * 512:(fb + 1) * 512],
                                     start=(e == 0), stop=(e == ET - 1))
                evict(VV[st][:, fb * 512:(fb + 1) * 512], ps)
        # Q^T[k] = W_q^T[:,k]^T xq^T : [128 kdim, 1024 own q rows]
        for qb in range(2):
            for k in range(KT):
                ps = pp.tile([P, 512], f32, name="ps_q", tag="pp")
                for e in range(ET):
                    nc.tensor.matmul(ps, wq[e][:, k * P:(k + 1) * P],
                                     xq[e][:, qb * 512:(qb + 1) * 512],
                                     start=(e == 0), stop=(e == ET - 1))
                evict(QT[k][:, qb * 512:(qb + 1) * 512], ps)

    # ---- attention phase
    pp_ctx.__exit__(None, None, None)
    acc_pool = ctx.enter_context(tc.tile_pool(name="acc", bufs=1))
    OACC = [acc_pool.tile([P, E], f32, name=f"oacc{t}", tag=f"oacc{t}") for t in range(NQT)]
    sp = ctx.enter_context(tc.tile_pool(name="sp", bufs=2, space="PSUM"))
    tp = ctx.enter_context(tc.tile_pool(name="tp", bufs=2, space="PSUM"))
    vp = ctx.enter_context(tc.tile_pool(name="vp", bufs=2, space="PSUM"))
    p_pool = ctx.enter_context(tc.tile_pool(name="ppb", bufs=4))
    pt_pool = ctx.enter_context(tc.tile_pool(name="ptp", bufs=6))

    def emit_pv(pend):
        # deferred transpose/copy/PV/accumulate for one (blk, t) work item;
        # runs one position behind the scores stream so the PE->copy->PE
        # transpose chain and exp latency hide behind matmul work.
        pb, w, blk, t = pend
        nst = w // P
        tpss = []
        for st in range(nst):
            tps = tp.tile([P, P], bf16, name="tps", tag="tp")
            nc.tensor.transpose(tps, pb[:, st * P:(st + 1) * P], ident)
            tpss.append(tps)
            if st > 0:
                pts = pt_pool.tile([P, P], bf16, name="pts", tag=f"pt{st-1}")
                if st % 2:
                    nc.vector.tensor_copy(pts, tpss[st - 1])
                else:
                    nc.scalar.copy(pts, tpss[st - 1])
                tpss[st - 1] = pts
        pts = pt_pool.tile([P, P], bf16, name="pts", tag=f"pt{nst-1}")
        if nst % 2:
            nc.vector.tensor_copy(pts, tpss[nst - 1])
        else:
            nc.scalar.copy(pts, tpss[nst - 1])
        tpss[nst - 1] = pts
        vps = [vp.tile([P, 512], f32, name=f"vps{fb}", tag=f"vp{fb}") for fb in range(2)]
        for st in range(nst):
            for fb in range(2):
                nc.tensor.matmul(vps[fb], tpss[st],
                                 VV[4 * blk + st][:, fb * 512:(fb + 1) * 512],
                                 start=(st == 0), stop=(st == nst - 1))
        for fb in range(2):
            dst = OACC[t][:, fb * 512:(fb + 1) * 512]
            if blk == 0:
                if fb == 0:
                    nc.vector.tensor_copy(dst, vps[fb])
                else:
                    nc.scalar.copy(dst, vps[fb])
            else:
                nc.vector.tensor_add(dst, dst, vps[fb])
        if blk == _n_blocks(t) - 1:
            nb = _n_blocks(t)
            rsum = fin_pool.tile([P, 1], f32, name="rsum", tag="rsum")
            nc.vector.reduce_sum(rsum, RS[t][:, :nb], axis=X)
            rinv = fin_pool.tile([P, 1], f32, name="rinv", tag="rinv")
            nc.vector.reciprocal(rinv, rsum)
            nc.scalar.activation(OACC[t], OACC[t],
                                 mybir.ActivationFunctionType.Copy, scale=rinv)
            nc.sync.dma_start(out=out_t[t], in_=OACC[t])

    pending = None  # pipeline carries across block boundaries
    for blk in range(NBLK):
        for t in range(2 * blk, NQT):
            w = min(512, 256 * (t + 1) - 512 * blk)
            is_diag = (blk == _n_blocks(t) - 1)
            sps = sp.tile([P, 512], f32, name="sps", tag="sp")
            for k in range(KT):
                nc.tensor.matmul(sps[:, :w], QT[k][:, t * P:(t + 1) * P],
                                 KTT[k][:, blk * 512: blk * 512 + w],
                                 start=(k == 0), stop=(k == KT - 1))
            if is_diag:
                nc.vector.tensor_add(sps[:, w - 256:w], sps[:, w - 256:w], cm)
            pb = p_pool.tile([P, 512], bf16, name="pb", tag="pb")
            nc.scalar.activation(pb[:, :w], sps[:, :w], Exp, scale=SCALE,
                                 accum_out=RS[t][:, blk:blk + 1])
            if pending is not None:
                emit_pv(pending)
            pending = (pb, w, blk, t)
    emit_pv(pending)


def build_program():
    if "nc" in _prog_cache:
        return _prog_cache["nc"]
    from contextlib import ExitStack
    from concourse import bacc, mybir
    import concourse.tile as tile

    nc = bacc.Bacc("TRN2", target_bir_lowering=False, debug=False,
                   num_devices=NCORES)
    f32 = mybir.dt.float32
    bf16 = mybir.dt.bfloat16
    ap = {
        "xT": nc.dram_tensor("xT", [E, S], bf16, kind="ExternalInput").ap(),
        "xTq": nc.dram_tensor("xTq", [E, 1024], bf16, kind="ExternalInput").ap(),
        "wqT": nc.dram_tensor("wqT", [E, KD], bf16, kind="ExternalInput").ap(),
        "wkT": nc.dram_tensor("wkT", [E, KD], bf16, kind="ExternalInput").ap(),
        "wvT": nc.dram_tensor("wvT", [E, E], bf16, kind="ExternalInput").ap(),
        "cmask": nc.dram_tensor("cmask", [P, 256], f32, kind="ExternalInput").ap(),
        "out": nc.dram_tensor("out", [1024, E], f32, kind="ExternalOutput").ap(),
    }
    with tile.TileContext(nc) as tc:
        with ExitStack() as ctx:
            _build_body(ctx, tc, ap)
    nc.compile()
    _prog_cache["nc"] = nc
    return nc


def make_in_maps(x, W_q, W_k, W_v):
    import ml_dtypes
    bf16 = ml_dtypes.bfloat16
    x = np.asarray(x, np.float32)
    wqT = np.ascontiguousarray(np.asarray(W_q, np.float32).T.astype(bf16))
    wkT = np.ascontiguousarray(np.asarray(W_k, np.float32).T.astype(bf16))
    wvT = np.ascontiguousarray(np.asarray(W_v, np.float32).T.astype(bf16))
    i = np.arange(P)[:, None]
    j = np.arange(256)[None, :]
    cmasks = [np.where(j <= i + 128, 0.0, NEG).astype(np.float32),
              np.where(j <= i, 0.0, NEG).astype(np.float32)]
    in_maps = []
    for c in range(NCORES):
        b, h = c // 2, c % 2
        xT = x[b].T.astype(bf16)
        qtiles = [2 * t + (1 - h) for t in range(NQT)]
        qcols = np.concatenate([np.arange(g * P, (g + 1) * P) for g in qtiles])
        xTq = np.ascontiguousarray(xT[:, qcols])
        in_maps.append({
            "xT": np.ascontiguousarray(xT), "xTq": xTq,
            "wqT": wqT, "wkT": wkT, "wvT": wvT,
            "cmask": cmasks[h],
        })
    return in_maps


def assemble(results):
    out = np.zeros((B, S, E), np.float32)
    for c in range(NCORES):
        b, h = c // 2, c % 2
        co = results[c]["out"]
        for t in range(NQT):
            g = 2 * t + (1 - h)
            out[b, g * P:(g + 1) * P, :] = co[t * P:(t + 1) * P]
    return out


def kernel(x, W_q, W_k, W_v):
    from concourse.bass_utils import run_bass_kernel_spmd
    nc = build_program()
    in_maps = make_in_maps(x, W_q, W_k, W_v)
    res = run_bass_kernel_spmd(nc, in_maps, core_ids=list(range(NCORES)))
    return assemble(res.results)


# revision 36
# speedup vs baseline: 2.5968x; 1.0516x over previous
"""Collective-free causal attention: 8 cores = 4 batches x 2 q-stripe sets.

Core c = (batch b = c//2, stripe set h = c%2) owns the 8 query stripes
g = 2t + (1-h), t in 0..7, of batch b.  Each core projects the FULL K^T and V
for its batch locally (duplicated within the pair) instead of exchanging
halves over a collective — the cost model charges intra-pair AllGathers like
inter-chip transfers (15us + size/40GBps, serialized), which dominated the
previous version; the duplicate K/V projection is ~55us of extra PE work vs
~210us+ of modeled collective time.

All matmul inputs are bf16 (host-converted); accumulation stays fp32 in PSUM,
softmax row sums and the output accumulator stay fp32; the output is staged
to bf16 for the store and widened to fp32 on the host.

DMA layout: inputs land as a few large 3D-AP transfers (batched across the
8 e-tiles) ordered so the K-projection's first PSUM group is runnable ~5us
in: x^T key-block 0 + W_k k-chunk 0 arrive first, then the remaining W_k
k-chunks land just ahead of the PE's k-loop consuming them.
"""

import numpy as np

B, S, E, KD = 4, 2048, 1024, 1024
NCORES = 8
P = 128
ET = E // P          # 8 e-tiles of the contraction dim
KT = KD // P         # 8 k-tiles of Q^T/K^T partition dim
NQT = 8              # 8 q stripes of 128 per core
NBLK = 4             # 4 key blocks of 512
NST = S // P         # 16 key subtiles of 128 (V tiles)
NEG = -30000.0
SCALE = 1.0 / float(np.sqrt(KD))

_prog_cache = {}


def _n_blocks(t):
    return (t + 2) // 2


def _build_body(ctx, tc, ap):
    from concourse import mybir
    from concourse.masks import make_identity

    nc = tc.nc
    f32 = mybir.dt.float32
    bf16 = mybir.dt.bfloat16
    Exp = mybir.ActivationFunctionType.Exp
    X = mybir.AxisListType.X

    # batched [partition, e, cols] views of the inputs
    xTb = ap["xT"].rearrange("(e p) s -> p e s", p=P)      # [128, 8, 2048]
    xqb = ap["xTq"].rearrange("(e p) q -> p e q", p=P)     # [128, 8, 1024]
    wqb = ap["wqT"].rearrange("(e p) k -> p e k", p=P)
    # W_k is host-shuffled to [k-chunk, p, e, c] so each k-chunk DMA moves
    # 2KB-contiguous rows (256B rows would pay the sub-512B descriptor
    # penalty on the head-critical path)
    wkb = ap["wkS"].rearrange("k p e c -> k p (e c)")      # [8, 128, 1024]
    wvb = ap["wvT"].rearrange("(e p) f -> p e f", p=P)
    out_t = ap["out"].rearrange("(t p) f -> t p f", p=P)

    # ---- persistent tiles
    qt_pool = ctx.enter_context(tc.tile_pool(name="qt", bufs=1))
    QT = [qt_pool.tile([P, 1024], bf16, name=f"qt{k}", tag=f"qt{k}") for k in range(KT)]
    kt_pool = ctx.enter_context(tc.tile_pool(name="ktp", bufs=1))
    KTT = [kt_pool.tile([P, S], bf16, name=f"ktt{k}", tag=f"ktt{k}") for k in range(KT)]
    vv_pool = ctx.enter_context(tc.tile_pool(name="vvp", bufs=1))
    VV = [vv_pool.tile([P, E], bf16, name=f"vv{s}", tag=f"vv{s}") for s in range(NST)]
    rs_pool = ctx.enter_context(tc.tile_pool(name="rsp", bufs=1))
    RS = [rs_pool.tile([P, NBLK], f32, name=f"rs{t}", tag=f"rs{t}") for t in range(NQT)]
    const_pool = ctx.enter_context(tc.tile_pool(name="const", bufs=1))
    fin_pool = ctx.enter_context(tc.tile_pool(name="fin", bufs=4))

    # PSUM plan: sp/tp live for the whole kernel; pp (projection evictions)
    # is scoped to the projection phase and its 2 banks are reused by the
    # attention vp pool (the handoff dependency lands on the first PV
    # matmuls, long after the last projection eviction — no stall).
    sp = ctx.enter_context(tc.tile_pool(name="sp", bufs=2, space="PSUM"))
    tp = ctx.enter_context(tc.tile_pool(name="tp", bufs=2, space="PSUM"))

    # PE warm-up: the cost model runs the PE at 1/3.7 speed for the first
    # ~100ns of a busy stretch and at half speed until 3us of continuous
    # activity.  Fill the input-DMA head (~7us) with throwaway matmuls on a
    # memset tile so every real matmul runs at full rate.
    warm_sb = const_pool.tile([P, 256], bf16, name="warm_sb")
    nc.gpsimd.memset(warm_sb, 0)
    for i in range(20):
        wps = sp.tile([P, 256], f32, name="wps", tag="sp")
        nc.tensor.matmul(wps, warm_sb[:, :P], warm_sb, start=True, stop=True)

    # GPSIMD cannot access PSUM, so evictions alternate DVE/ACT.
    evict_ops = [lambda d, s: nc.vector.tensor_copy(d, s),
                 lambda d, s: nc.scalar.copy(d, s)]
    evict_i = 0

    def evict(dst, src):
        nonlocal evict_i
        evict_ops[evict_i % 2](dst, src)
        evict_i += 1

    # ---- projection phase (scoped input pools + scoped eviction PSUM pool)
    with tc.tile_pool(name="xtp", bufs=1) as xt_pool, \
         tc.tile_pool(name="xqp", bufs=1) as xq_pool, \
         tc.tile_pool(name="wqp", bufs=1) as wq_pool, \
         tc.tile_pool(name="wkp", bufs=1) as wk_pool, \
         tc.tile_pool(name="wvp", bufs=1) as wv_pool, \
         tc.tile_pool(name="pp", bufs=2, space="PSUM") as pp:
        # x^T per key block sb: [p, (e 512)]  (slice e: [:, e*512:(e+1)*512])
        xts = [xt_pool.tile([P, ET * 512], bf16, name=f"xts{sb}", tag=f"xts{sb}")
               for sb in range(NBLK)]
        # W_k^T per k-chunk: [p, (e 128)]
        wkc = [wk_pool.tile([P, ET * P], bf16, name=f"wkc{k}", tag=f"wkc{k}")
               for k in range(KT)]
        wq_all = wq_pool.tile([P, ET * KD], bf16, name="wq", tag="wq")
        xq_all = xq_pool.tile([P, ET * 1024], bf16, name="xq", tag="xq")
        wv_all = wv_pool.tile([P, ET * E], bf16, name="wv", tag="wv")

        # DMA order tuned so the PE's first PSUM group is runnable ~5us in
        # and later chunks land just ahead of consumption.
        # first key block of x^T lands in two halves so the K projection's
        # e-accumulation can start after ~half the transfer
        xts0v = xts[0].rearrange("p (e s) -> p e s", s=512)
        nc.sync.dma_start(out=xts0v[:, 0:4, :], in_=xTb[:, 0:4, 0:512])
        nc.sync.dma_start(out=wkc[0], in_=wkb[0])
        nc.sync.dma_start(out=xts0v[:, 4:8, :], in_=xTb[:, 4:8, 0:512])
        for k in range(1, KT):
            nc.sync.dma_start(out=wkc[k], in_=wkb[k])
        for sb in range(1, NBLK):
            nc.sync.dma_start(
                out=xts[sb].rearrange("p (e s) -> p e s", s=512),
                in_=xTb[:, :, sb * 512:(sb + 1) * 512])
        nc.sync.dma_start(
            out=wq_all.rearrange("p (e k) -> p e k", k=KD), in_=wqb)
        nc.sync.dma_start(
            out=xq_all.rearrange("p (e q) -> p e q", q=1024), in_=xqb)
        nc.sync.dma_start(
            out=wv_all.rearrange("p (e f) -> p e f", f=E), in_=wvb)
        cm = const_pool.tile([P, 256], f32, name="cm")
        nc.sync.dma_start(out=cm, in_=ap["cmask"])

        # K^T[k] = W_k^T[:,k]^T x^T : [128 kdim, 2048 keys]
        for sb in range(NBLK):
            for k in range(KT):
                ps = pp.tile([P, 512], f32, name="ps_k", tag="pp")
                for e in range(ET):
                    nc.tensor.matmul(ps, wkc[k][:, e * P:(e + 1) * P],
                                     xts[sb][:, e * 512:(e + 1) * 512],
                                     start=(e == 0), stop=(e == ET - 1))
                evict(KTT[k][:, sb * 512:(sb + 1) * 512], ps)
        # Q^T[k] = W_q^T[:,k]^T xq^T : [128 kdim, 1024 own q rows]
        for qb in range(2):
            for k in range(KT):
                ps = pp.tile([P, 512], f32, name="ps_q", tag="pp")
                for e in range(ET):
                    nc.tensor.matmul(
                        ps, wq_all[:, e * KD + k * P: e * KD + (k + 1) * P],
                        xq_all[:, e * 1024 + qb * 512: e * 1024 + (qb + 1) * 512],
                        start=(e == 0), stop=(e == ET - 1))
                evict(QT[k][:, qb * 512:(qb + 1) * 512], ps)
        # V[st] = x[st rows] W_v^T : [128 keys, 1024 features]
        for st in range(NST):
            sb, stv = st // 4, st % 4
            for fb in range(2):
                ps = pp.tile([P, 512], f32, name="ps_v", tag="pp")
                for e in range(ET):
                    nc.tensor.matmul(
                        ps, xts[sb][:, e * 512 + stv * P: e * 512 + (stv + 1) * P],
                        wv_all[:, e * E + fb * 512: e * E + (fb + 1) * 512],
                        start=(e == 0), stop=(e == ET - 1))
                evict(VV[st][:, fb * 512:(fb + 1) * 512], ps)

        ident_f32 = const_pool.tile([P, P], f32, name="ident_f32")
        make_identity(nc, ident_f32)
        ident = const_pool.tile([P, P], bf16, name="ident")
        nc.vector.tensor_copy(ident, ident_f32)

    # ---- attention phase
    vp = ctx.enter_context(tc.tile_pool(name="vp", bufs=2, space="PSUM"))
    acc_pool = ctx.enter_context(tc.tile_pool(name="acc", bufs=1))
    # OACC for the last-finalized stripes (t=6,7) is bf16 so the final block
    # can fold it into the PV PSUM via an identity matmul and scale straight
    # out of PSUM — this shortens the end-of-kernel dependency tail.
    OACC = [acc_pool.tile([P, E], bf16 if t >= 6 else f32,
                          name=f"oacc{t}", tag=f"oacc{t}") for t in range(NQT)]
    p_pool = ctx.enter_context(tc.tile_pool(name="ppb", bufs=4))
    pt_pool = ctx.enter_context(tc.tile_pool(name="ptp", bufs=6))

    def emit_pv(pend):
        # deferred transpose/copy/PV/accumulate for one (blk, t) work item;
        # runs one position behind the scores stream so the PE->copy->PE
        # transpose chain and exp latency hide behind matmul work.
        pb, w, blk, t = pend
        nst = w // P
        tpss = []
        for st in range(nst):
            tps = tp.tile([P, P], bf16, name="tps", tag="tp")
            nc.tensor.transpose(tps, pb[:, st * P:(st + 1) * P], ident)
            tpss.append(tps)
            if st > 0:
                pts = pt_pool.tile([P, P], bf16, name="pts", tag=f"pt{st-1}")
                if st % 2:
                    nc.vector.tensor_copy(pts, tpss[st - 1])
                else:
                    nc.scalar.copy(pts, tpss[st - 1])
                tpss[st - 1] = pts
        pts = pt_pool.tile([P, P], bf16, name="pts", tag=f"pt{nst-1}")
        if nst % 2:
            nc.vector.tensor_copy(pts, tpss[nst - 1])
        else:
            nc.scalar.copy(pts, tpss[nst - 1])
        tpss[nst - 1] = pts
        is_final = (blk == _n_blocks(t) - 1)
        # Stripes scheduled at the very end of the kernel (t=6,7 and the
        # single-block t=0,1) finish straight out of PSUM to keep the
        # end-of-kernel dependency tail short.
        psum_fin = is_final and (t >= 6 or t <= 1)
        vps = [vp.tile([P, 512], f32, name=f"vps{fb}", tag=f"vp{fb}") for fb in range(2)]
        fold = psum_fin and blk > 0
        for st in range(nst):
            for fb in range(2):
                nc.tensor.matmul(vps[fb], tpss[st],
                                 VV[4 * blk + st][:, fb * 512:(fb + 1) * 512],
                                 start=(st == 0),
                                 stop=(st == nst - 1) and not fold)
        if psum_fin:
            if fold:
                # fold the accumulated output into the PV PSUM on the PE
                for fb in range(2):
                    nc.tensor.matmul(vps[fb], ident,
                                     OACC[t][:, fb * 512:(fb + 1) * 512],
                                     start=False, stop=True)
        else:
            for fb in range(2):
                dst = OACC[t][:, fb * 512:(fb + 1) * 512]
                if blk == 0:
                    if fb == 0:
                        nc.vector.tensor_copy(dst, vps[fb])
                    else:
                        nc.scalar.copy(dst, vps[fb])
                else:
                    nc.vector.tensor_add(dst, dst, vps[fb])
        if is_final:
            nb = _n_blocks(t)
            rsum = fin_pool.tile([P, 1], f32, name="rsum", tag="rsum")
            nc.vector.reduce_sum(rsum, RS[t][:, :nb], axis=X)
            rinv = fin_pool.tile([P, 1], f32, name="rinv", tag="rinv")
            nc.vector.reciprocal(rinv, rsum)
            obf = fin_pool.tile([P, E], bf16, name="obf", tag="obf", bufs=4)
            if psum_fin:
                # scale the two halves on different engines in parallel,
                # then store with a single DMA (HWDGE overhead is per-DMA)
                nc.scalar.activation(obf[:, 0:512], vps[0],
                                     mybir.ActivationFunctionType.Copy,
                                     scale=rinv)
                nc.vector.tensor_scalar_mul(obf[:, 512:1024], vps[1], rinv)
                nc.sync.dma_start(out=out_t[t], in_=obf)
            else:
                nc.scalar.activation(obf, OACC[t],
                                     mybir.ActivationFunctionType.Copy,
                                     scale=rinv)
                nc.sync.dma_start(out=out_t[t], in_=obf)

    # Schedule: blocks in order, t descending within each block.  The two
    # single-block stripes (t=0,1) are split: scores/exp/transpose run FIRST
    # (they only need Q^T and key block 0), holding P^T and 1/rowsum in
    # SBUF; only their PV matmuls + PSUM-direct scale + store run at the
    # very end, so the kernel tail has no exp/transpose latency in it.
    items = []
    for blk in range(NBLK):
        for t in range(NQT - 1, max(2 * blk, 2) - 1, -1):
            items.append((t, blk))

    held = {}  # t -> (pts tiles, rinv)
    for t in (1, 0):
        w = 256 * (t + 1)
        sps = sp.tile([P, 512], f32, name="sps", tag="sp")
        for k in range(KT):
            nc.tensor.matmul(sps[:, :w], QT[k][:, t * P:(t + 1) * P],
                             KTT[k][:, :w], start=(k == 0), stop=(k == KT - 1))
        nc.vector.tensor_add(sps[:, w - 256:w], sps[:, w - 256:w], cm)
        pb = p_pool.tile([P, 512], bf16, name="pb", tag="pb")
        nc.scalar.activation(pb[:, :w], sps[:, :w], Exp, scale=SCALE,
                             accum_out=RS[t][:, 0:1])
        ptss = []
        for st in range(w // P):
            tps = tp.tile([P, P], bf16, name="tps", tag="tp")
            nc.tensor.transpose(tps, pb[:, st * P:(st + 1) * P], ident)
            pts = fin_pool.tile([P, P], bf16, name="hpt", tag=f"hpt{t}_{st}",
                                bufs=1)
            if st % 2:
                nc.vector.tensor_copy(pts, tps)
            else:
                nc.scalar.copy(pts, tps)
            ptss.append(pts)
        rinv = fin_pool.tile([P, 1], f32, name="hri", tag=f"hri{t}", bufs=1)
        nc.vector.reciprocal(rinv, RS[t][:, 0:1])
        held[t] = (ptss, rinv)

    pending = None  # pipeline carries across block boundaries
    for t, blk in items:
            w = min(512, 256 * (t + 1) - 512 * blk)
            is_diag = (blk == _n_blocks(t) - 1)
            sps = sp.tile([P, 512], f32, name="sps", tag="sp")
            for k in range(KT):
                nc.tensor.matmul(sps[:, :w], QT[k][:, t * P:(t + 1) * P],
                                 KTT[k][:, blk * 512: blk * 512 + w],
                                 start=(k == 0), stop=(k == KT - 1))
            if is_diag:
                nc.vector.tensor_add(sps[:, w - 256:w], sps[:, w - 256:w], cm)
            pb = p_pool.tile([P, 512], bf16, name="pb", tag="pb")
            nc.scalar.activation(pb[:, :w], sps[:, :w], Exp, scale=SCALE,
                                 accum_out=RS[t][:, blk:blk + 1])
            if pending is not None:
                emit_pv(pending)
            pending = (pb, w, blk, t)

    def emit_held_pv(t):
        # PV + PSUM-direct scale + store from pre-computed P^T and 1/rowsum
        ptss, rinv = held[t]
        vps = [vp.tile([P, 512], f32, name=f"vps{fb}", tag=f"vp{fb}")
               for fb in range(2)]
        for st in range(len(ptss)):
            for fb in range(2):
                nc.tensor.matmul(vps[fb], ptss[st],
                                 VV[st][:, fb * 512:(fb + 1) * 512],
                                 start=(st == 0), stop=(st == len(ptss) - 1))
        obf = fin_pool.tile([P, E], bf16, name="obf", tag="obf", bufs=4)
        nc.scalar.activation(obf[:, 0:512], vps[0],
                             mybir.ActivationFunctionType.Copy, scale=rinv)
        nc.vector.tensor_scalar_mul(obf[:, 512:1024], vps[1], rinv)
        nc.sync.dma_start(out=out_t[t], in_=obf)

    # interleave the held stripes' PVs so each store chain hides under the
    # next stripe's PE work; only the very last store is exposed.
    emit_held_pv(1)
    emit_pv(pending)
    emit_held_pv(0)


def build_program():
    if "nc" in _prog_cache:
        return _prog_cache["nc"]
    from contextlib import ExitStack
    from concourse import bacc, mybir
    import concourse.tile as tile

    nc = bacc.Bacc("TRN2", target_bir_lowering=False, debug=False,
                   num_devices=NCORES)
    f32 = mybir.dt.float32
    bf16 = mybir.dt.bfloat16
    ap = {
        "xT": nc.dram_tensor("xT", [E, S], bf16, kind="ExternalInput").ap(),
        "xTq": nc.dram_tensor("xTq", [E, 1024], bf16, kind="ExternalInput").ap(),
        "wqT": nc.dram_tensor("wqT", [E, KD], bf16, kind="ExternalInput").ap(),
        "wkS": nc.dram_tensor("wkS", [KT, P, ET, P], bf16,
                              kind="ExternalInput").ap(),
        "wvT": nc.dram_tensor("wvT", [E, E], bf16, kind="ExternalInput").ap(),
        "cmask": nc.dram_tensor("cmask", [P, 256], f32, kind="ExternalInput").ap(),
        "out": nc.dram_tensor("out", [1024, E], bf16, kind="ExternalOutput").ap(),
    }
    with tile.TileContext(nc) as tc:
        with ExitStack() as ctx:
            _build_body(ctx, tc, ap)
    nc.compile()
    _prog_cache["nc"] = nc
    return nc


def make_in_maps(x, W_q, W_k, W_v):
    import ml_dtypes
    bf16 = ml_dtypes.bfloat16
    x = np.asarray(x, np.float32)
    wqT = np.ascontiguousarray(np.asarray(W_q, np.float32).T.astype(bf16))
    wkT = np.asarray(W_k, np.float32).T.astype(bf16)
    # [k-chunk, p, e, c]: wkS[k, p, e, c] = wkT[e*128+p, k*128+c]
    wkS = np.ascontiguousarray(
        wkT.reshape(ET, P, KT, P).transpose(2, 1, 0, 3))
    wvT = np.ascontiguousarray(np.asarray(W_v, np.float32).T.astype(bf16))
    i = np.arange(P)[:, None]
    j = np.arange(256)[None, :]
    cmasks = [np.where(j <= i + 128, 0.0, NEG).astype(np.float32),
              np.where(j <= i, 0.0, NEG).astype(np.float32)]
    in_maps = []
    for c in range(NCORES):
        b, h = c // 2, c % 2
        xT = x[b].T.astype(bf16)
        qtiles = [2 * t + (1 - h) for t in range(NQT)]
        qcols = np.concatenate([np.arange(g * P, (g + 1) * P) for g in qtiles])
        xTq = np.ascontiguousarray(xT[:, qcols])
        in_maps.append({
            "xT": np.ascontiguousarray(xT), "xTq": xTq,
            "wqT": wqT, "wkS": wkS, "wvT": wvT,
            "cmask": cmasks[h],
        })
    return in_maps


def assemble(results):
    out = np.zeros((B, S, E), np.float32)
    for c in range(NCORES):
        b, h = c // 2, c % 2
        co = np.asarray(results[c]["out"], dtype=np.float32)
        for t in range(NQT):
            g = 2 * t + (1 - h)
            out[b, g * P:(g + 1) * P, :] = co[t * P:(t + 1) * P]
    return out


def kernel(x, W_q, W_k, W_v):
    from concourse.bass_utils import run_bass_kernel_spmd
    nc = build_program()
    in_maps = make_in_maps(x, W_q, W_k, W_v)
    res = run_bass_kernel_spmd(nc, in_maps, core_ids=list(range(NCORES)))
    return assemble(res.results)


# revision 57
# speedup vs baseline: 2.7159x; 1.0459x over previous
"""Collective-free causal attention: 8 cores = 4 batches x 2 q-stripe sets.

Core c = (batch b = c//2, stripe set h = c%2) owns the 8 query stripes
g = 2t + (1-h), t in 0..7, of batch b.  Each core projects the FULL K^T and V
for its batch locally (duplicated within the pair) instead of exchanging
halves over a collective — the cost model charges intra-pair AllGathers like
inter-chip transfers (15us + size/40GBps, serialized), which dominated the
previous version; the duplicate K/V projection is ~55us of extra PE work vs
~210us+ of modeled collective time.

All matmul inputs are bf16 (host-converted); accumulation stays fp32 in PSUM,
softmax row sums and the output accumulator stay fp32; the output is staged
to bf16 for the store and widened to fp32 on the host.

DMA layout: inputs land as a few large 3D-AP transfers (batched across the
8 e-tiles) ordered so the K-projection's first PSUM group is runnable ~5us
in: x^T key-block 0 + W_k k-chunk 0 arrive first, then the remaining W_k
k-chunks land just ahead of the PE's k-loop consuming them.
"""

import numpy as np

B, S, E, KD = 4, 2048, 1024, 1024
NCORES = 8
P = 128
ET = E // P          # 8 e-tiles of the contraction dim
KT = KD // P         # 8 k-tiles of Q^T/K^T partition dim
NQT = 8              # 8 q stripes of 128 per core
NBLK = 4             # 4 key blocks of 512
NST = S // P         # 16 key subtiles of 128 (V tiles)
NEG = -30000.0
SCALE = 1.0 / float(np.sqrt(KD))

_prog_cache = {}


def _n_blocks(t):
    return (t + 2) // 2


def _build_body(ctx, tc, ap):
    from concourse import mybir
    from concourse.masks import make_identity

    nc = tc.nc
    f32 = mybir.dt.float32
    bf16 = mybir.dt.bfloat16
    Exp = mybir.ActivationFunctionType.Exp
    X = mybir.AxisListType.X

    # batched [partition, e, cols] views of the inputs
    xTb = ap["xT"].rearrange("(e p) s -> p e s", p=P)      # [128, 8, 2048]
    xqb = ap["xTq"].rearrange("(e p) q -> p e q", p=P)     # [128, 8, 1024]
    wqb = ap["wqT"].rearrange("(e p) k -> p e k", p=P)
    # W_k is host-shuffled to [k-chunk, p, e, c] so each k-chunk DMA moves
    # 2KB-contiguous rows (256B rows would pay the sub-512B descriptor
    # penalty on the head-critical path)
    wkb = ap["wkS"].rearrange("k p e c -> k p (e c)")      # [8, 128, 1024]
    wvb = ap["wvT"].rearrange("(e p) f -> p e f", p=P)
    out_t = ap["out"].rearrange("(t p) f -> t p f", p=P)

    # ---- persistent tiles
    qt_pool = ctx.enter_context(tc.tile_pool(name="qt", bufs=1))
    QT = [qt_pool.tile([P, 1024], bf16, name=f"qt{k}", tag=f"qt{k}") for k in range(KT)]
    kt_pool = ctx.enter_context(tc.tile_pool(name="ktp", bufs=1))
    KTT = [kt_pool.tile([P, S], bf16, name=f"ktt{k}", tag=f"ktt{k}") for k in range(KT)]
    vv_pool = ctx.enter_context(tc.tile_pool(name="vvp", bufs=1))
    VV = [vv_pool.tile([P, E], bf16, name=f"vv{s}", tag=f"vv{s}") for s in range(NST)]
    rs_pool = ctx.enter_context(tc.tile_pool(name="rsp", bufs=1))
    RS = [rs_pool.tile([P, NBLK], f32, name=f"rs{t}", tag=f"rs{t}") for t in range(NQT)]
    const_pool = ctx.enter_context(tc.tile_pool(name="const", bufs=1))
    fin_pool = ctx.enter_context(tc.tile_pool(name="fin", bufs=4))

    # PSUM plan: sp lives for the whole kernel; pp (projection evictions,
    # 2 banks) is scoped to the projection phase and its banks are reused
    # by the attention vp pool (2 tags x 3 bufs = 6 banks; the handoff
    # dependency lands on the first PV matmuls, long after the last
    # projection eviction — no stall).
    sp = ctx.enter_context(tc.tile_pool(name="sp", bufs=2, space="PSUM"))

    # PE warm-up: the cost model runs the PE at 1/3.7 speed for the first
    # ~100ns of a busy stretch and at half speed until 3us of continuous
    # activity.  Fill the input-DMA head (~7us) with throwaway matmuls on a
    # memset tile so every real matmul runs at full rate.
    warm_sb = const_pool.tile([P, 256], bf16, name="warm_sb")
    nc.gpsimd.memset(warm_sb, 0)
    for i in range(22):
        wps = sp.tile([P, 256], f32, name="wps", tag="sp")
        nc.tensor.matmul(wps, warm_sb[:, :P], warm_sb, start=True, stop=True)

    p_pool = ctx.enter_context(tc.tile_pool(name="ppb", bufs=4))

    # GPSIMD cannot access PSUM, so evictions alternate DVE/ACT.
    evict_ops = [lambda d, s: nc.vector.tensor_copy(d, s),
                 lambda d, s: nc.scalar.copy(d, s)]
    evict_i = 0

    def evict(dst, src):
        nonlocal evict_i
        evict_ops[evict_i % 2](dst, src)
        evict_i += 1

    # ---- projection phase (scoped input pools + scoped eviction PSUM pool)
    held = {}  # t -> (P^T tiles, 1/rowsum) for the split stripes t=0,1
    with tc.tile_pool(name="xtp", bufs=1) as xt_pool, \
         tc.tile_pool(name="xqp", bufs=1) as xq_pool, \
         tc.tile_pool(name="wqp", bufs=1) as wq_pool, \
         tc.tile_pool(name="wkp", bufs=1) as wk_pool, \
         tc.tile_pool(name="wvp", bufs=1) as wv_pool, \
         tc.tile_pool(name="pp", bufs=2, space="PSUM") as pp:
        # x^T per key block sb: [p, (e 512)]  (slice e: [:, e*512:(e+1)*512])
        xts = [xt_pool.tile([P, ET * 512], bf16, name=f"xts{sb}", tag=f"xts{sb}")
               for sb in range(NBLK)]
        # W_k^T per k-chunk: [p, (e 128)]
        wkc = [wk_pool.tile([P, ET * P], bf16, name=f"wkc{k}", tag=f"wkc{k}")
               for k in range(KT)]
        wq_all = wq_pool.tile([P, ET * KD], bf16, name="wq", tag="wq")
        xq_all = xq_pool.tile([P, ET * 1024], bf16, name="xq", tag="xq")
        wv_all = wv_pool.tile([P, ET * E], bf16, name="wv", tag="wv")

        # DMA order tuned so the PE's first PSUM group is runnable ~5us in
        # and later chunks land just ahead of consumption.
        # first key block of x^T lands in two halves so the K projection's
        # e-accumulation can start after ~half the transfer
        xts0v = xts[0].rearrange("p (e s) -> p e s", s=512)
        nc.sync.dma_start(out=xts0v[:, 0:4, :], in_=xTb[:, 0:4, 0:512])
        nc.sync.dma_start(out=wkc[0], in_=wkb[0])
        nc.sync.dma_start(out=xts0v[:, 4:6, :], in_=xTb[:, 4:6, 0:512])
        nc.sync.dma_start(out=xts0v[:, 6:8, :], in_=xTb[:, 6:8, 0:512])
        for k in range(1, KT):
            nc.sync.dma_start(out=wkc[k], in_=wkb[k])
        for sb in range(1, NBLK):
            nc.sync.dma_start(
                out=xts[sb].rearrange("p (e s) -> p e s", s=512),
                in_=xTb[:, :, sb * 512:(sb + 1) * 512])
        nc.sync.dma_start(
            out=wq_all.rearrange("p (e k) -> p e k", k=KD), in_=wqb)
        nc.sync.dma_start(
            out=xq_all.rearrange("p (e q) -> p e q", q=1024), in_=xqb)
        nc.sync.dma_start(
            out=wv_all.rearrange("p (e f) -> p e f", f=E), in_=wvb)
        cm = const_pool.tile([P, 256], f32, name="cm")
        nc.sync.dma_start(out=cm, in_=ap["cmask"])

        # K^T[k] = W_k^T[:,k]^T x^T : [128 kdim, 2048 keys]
        for sb in range(NBLK):
            for k in range(KT):
                ps = pp.tile([P, 512], f32, name="ps_k", tag="pp")
                for e in range(ET):
                    nc.tensor.matmul(ps, wkc[k][:, e * P:(e + 1) * P],
                                     xts[sb][:, e * 512:(e + 1) * 512],
                                     start=(e == 0), stop=(e == ET - 1))
                evict(KTT[k][:, sb * 512:(sb + 1) * 512], ps)
        # Q^T[k] = W_q^T[:,k]^T xq^T : [128 kdim, 1024 own q rows]
        for qb in range(2):
            for k in range(KT):
                ps = pp.tile([P, 512], f32, name="ps_q", tag="pp")
                for e in range(ET):
                    nc.tensor.matmul(
                        ps, wq_all[:, e * KD + k * P: e * KD + (k + 1) * P],
                        xq_all[:, e * 1024 + qb * 512: e * 1024 + (qb + 1) * 512],
                        start=(e == 0), stop=(e == ET - 1))
                evict(QT[k][:, qb * 512:(qb + 1) * 512], ps)

        # scores/exp/transpose for the single-block stripes t=0,1 run here,
        # between the Q and V projections (their inputs — Q^T and key block
        # 0 — are ready); P^T and 1/rowsum are held in SBUF and only their
        # PV + store run at the very end of the kernel.
        for t in (1, 0):
            w = 256 * (t + 1)
            sps = sp.tile([P, 512], f32, name="sps", tag="sp")
            for k in range(KT):
                nc.tensor.matmul(sps[:, :w], QT[k][:, t * P:(t + 1) * P],
                                 KTT[k][:, :w], start=(k == 0),
                                 stop=(k == KT - 1))
            nc.vector.tensor_add(sps[:, w - 256:w], sps[:, w - 256:w], cm)
            pb = p_pool.tile([P, 512], bf16, name="pb", tag="pb")
            nc.scalar.activation(pb[:, :w], sps[:, :w], Exp, scale=SCALE,
                                 accum_out=RS[t][:, 0:1])
            ptile = fin_pool.tile([P, w], bf16, name="hpt", tag=f"hpt{t}",
                                  bufs=1)
            nc.sync.dma_start_transpose(
                out=ptile.rearrange("p (st c) -> p st c", c=P),
                in_=pb[:, :w])
            rinv = fin_pool.tile([P, 1], f32, name="hri", tag=f"hri{t}",
                                 bufs=1)
            nc.vector.reciprocal(rinv, RS[t][:, 0:1])
            held[t] = (ptile, w // P, rinv)

        # V[st] = x[st rows] W_v^T : [128 keys, 1024 features]
        for st in range(NST):
            sb, stv = st // 4, st % 4
            for fb in range(2):
                ps = pp.tile([P, 512], f32, name="ps_v", tag="pp")
                for e in range(ET):
                    nc.tensor.matmul(
                        ps, xts[sb][:, e * 512 + stv * P: e * 512 + (stv + 1) * P],
                        wv_all[:, e * E + fb * 512: e * E + (fb + 1) * 512],
                        start=(e == 0), stop=(e == ET - 1))
                evict(VV[st][:, fb * 512:(fb + 1) * 512], ps)

    # ---- attention phase
    vp = ctx.enter_context(tc.tile_pool(name="vp", bufs=2, space="PSUM"))
    pt_pool = ctx.enter_context(tc.tile_pool(name="ptp", bufs=5))

    # Attention is stripe-major: all of K^T/V is resident in SBUF, so each
    # stripe walks its key blocks back-to-back and accumulates PV entirely
    # in PSUM (vps holds both feature halves, one PSUM accumulation group
    # spanning the stripe's blocks).  No SBUF output accumulator, no DVE
    # adds, and every stripe finishes straight out of PSUM.  Only two
    # stripes' PV groups are ever in flight (vp tags x bufs=2 = 4 banks).
    cur_vps = {}
    fin_parity = [0]

    def scale_out(t, vps, rinv):
        obf = fin_pool.tile([P, E], bf16, name="obf", tag="obf", bufs=4)
        # scale the two halves on different engines in parallel (alternating
        # the assignment between consecutive finalizes so back-to-back
        # stripe finishes don't queue on one engine), then store with a
        # single DMA (HWDGE overhead is per-DMA)
        halves = [(0, vps[0]), (1, vps[1])]
        if fin_parity[0]:
            halves.reverse()
        fin_parity[0] ^= 1
        for i, (fb, src) in enumerate(halves):
            dst = obf[:, fb * 512:(fb + 1) * 512]
            if i == 0:
                nc.scalar.activation(dst, src,
                                     mybir.ActivationFunctionType.Copy,
                                     scale=rinv)
            else:
                nc.vector.tensor_scalar_mul(dst, src, rinv)
        nc.sync.dma_start(out=out_t[t], in_=obf)

    def finalize(t, vps):
        rsum = fin_pool.tile([P, 1], f32, name="rsum", tag="rsum")
        nc.vector.reduce_sum(rsum, RS[t][:, :_n_blocks(t)], axis=X)
        rinv = fin_pool.tile([P, 1], f32, name="rinv", tag="rinv")
        nc.vector.reciprocal(rinv, rsum)
        scale_out(t, vps, rinv)

    def emit_pv(pend):
        # deferred PV for one (t, blk) work item; P^T arrives via an async
        # DMA transpose issued right after the exp, two positions ahead, so
        # its ~3us flight time hides behind the scores stream.
        ptile, w, blk, t = pend
        nst = w // P
        if blk == 0:
            cur_vps[t] = [vp.tile([P, 512], f32, name=f"vps{fb}",
                                  tag=f"vp{fb}") for fb in range(2)]
        vps = cur_vps[t]
        is_final = (blk == _n_blocks(t) - 1)
        for st in range(nst):
            for fb in range(2):
                nc.tensor.matmul(vps[fb], ptile[:, st * P:(st + 1) * P],
                                 VV[4 * blk + st][:, fb * 512:(fb + 1) * 512],
                                 start=(blk == 0 and st == 0),
                                 stop=is_final and (st == nst - 1))
        if is_final:
            finalize(t, vps)

    # stripe-major schedule: big stripes first; the held single-block
    # stripes t=1,0 (scores pre-computed during the projection phase) come
    # last, so the kernel tail is just PV -> scale -> store.
    from collections import deque
    pend_q = deque()
    for t in (7, 3, 6, 2, 5, 4):
        for blk in range(_n_blocks(t)):
            w = min(512, 256 * (t + 1) - 512 * blk)
            is_diag = (blk == _n_blocks(t) - 1)
            sps = sp.tile([P, 512], f32, name="sps", tag="sp")
            for k in range(KT):
                nc.tensor.matmul(sps[:, :w], QT[k][:, t * P:(t + 1) * P],
                                 KTT[k][:, blk * 512: blk * 512 + w],
                                 start=(k == 0), stop=(k == KT - 1))
            if is_diag:
                nc.vector.tensor_add(sps[:, w - 256:w], sps[:, w - 256:w], cm)
            pb = p_pool.tile([P, 512], bf16, name="pb", tag="pb")
            nc.scalar.activation(pb[:, :w], sps[:, :w], Exp, scale=SCALE,
                                 accum_out=RS[t][:, blk:blk + 1])
            ptile = pt_pool.tile([P, 512], bf16, name="ptd", tag="ptd")
            nc.sync.dma_start_transpose(
                out=ptile.rearrange("p (st c) -> p st c", c=P)[:, :w // P, :],
                in_=pb[:, :w])
            pend_q.append((ptile, w, blk, t))
            if len(pend_q) > 3:
                emit_pv(pend_q.popleft())

    def emit_held_pv(t):
        # PV + PSUM-direct scale + store from pre-computed P^T and 1/rowsum
        ptile, nst, rinv = held[t]
        vps = [vp.tile([P, 512], f32, name=f"vps{fb}", tag=f"vp{fb}")
               for fb in range(2)]
        for st in range(nst):
            for fb in range(2):
                nc.tensor.matmul(vps[fb], ptile[:, st * P:(st + 1) * P],
                                 VV[st][:, fb * 512:(fb + 1) * 512],
                                 start=(st == 0), stop=(st == nst - 1))
        scale_out(t, vps, rinv)

    # drain: interleave the held stripes' PVs so each store chain hides
    # under the next stripe's PE work and the last in-flight DMA transpose
    # gets PE work to hide behind; only the very last store is exposed.
    emit_pv(pend_q.popleft())
    emit_pv(pend_q.popleft())
    emit_held_pv(1)
    emit_pv(pend_q.popleft())
    emit_held_pv(0)


def build_program():
    if "nc" in _prog_cache:
        return _prog_cache["nc"]
    from contextlib import ExitStack
    from concourse import bacc, mybir
    import concourse.tile as tile

    nc = bacc.Bacc("TRN2", target_bir_lowering=False, debug=False,
                   num_devices=NCORES)
    f32 = mybir.dt.float32
    bf16 = mybir.dt.bfloat16
    ap = {
        "xT": nc.dram_tensor("xT", [E, S], bf16, kind="ExternalInput").ap(),
        "xTq": nc.dram_tensor("xTq", [E, 1024], bf16, kind="ExternalInput").ap(),
        "wqT": nc.dram_tensor("wqT", [E, KD], bf16, kind="ExternalInput").ap(),
        "wkS": nc.dram_tensor("wkS", [KT, P, ET, P], bf16,
                              kind="ExternalInput").ap(),
        "wvT": nc.dram_tensor("wvT", [E, E], bf16, kind="ExternalInput").ap(),
        "cmask": nc.dram_tensor("cmask", [P, 256], f32, kind="ExternalInput").ap(),
        "out": nc.dram_tensor("out", [1024, E], bf16, kind="ExternalOutput").ap(),
    }
    with tile.TileContext(nc) as tc:
        with ExitStack() as ctx:
            _build_body(ctx, tc, ap)
    nc.compile()
    _prog_cache["nc"] = nc
    return nc


def make_in_maps(x, W_q, W_k, W_v):
    import ml_dtypes
    bf16 = ml_dtypes.bfloat16
    x = np.asarray(x, np.float32)
    wqT = np.ascontiguousarray(np.asarray(W_q, np.float32).T.astype(bf16))
    wkT = np.asarray(W_k, np.float32).T.astype(bf16)
    # [k-chunk, p, e, c]: wkS[k, p, e, c] = wkT[e*128+p, k*128+c]
    wkS = np.ascontiguousarray(
        wkT.reshape(ET, P, KT, P).transpose(2, 1, 0, 3))
    wvT = np.ascontiguousarray(np.asarray(W_v, np.float32).T.astype(bf16))
    i = np.arange(P)[:, None]
    j = np.arange(256)[None, :]
    cmasks = [np.where(j <= i + 128, 0.0, NEG).astype(np.float32),
              np.where(j <= i, 0.0, NEG).astype(np.float32)]
    in_maps = []
    for c in range(NCORES):
        b, h = c // 2, c % 2
        xT = x[b].T.astype(bf16)
        qtiles = [2 * t + (1 - h) for t in range(NQT)]
        qcols = np.concatenate([np.arange(g * P, (g + 1) * P) for g in qtiles])
        xTq = np.ascontiguousarray(xT[:, qcols])
        in_maps.append({
            "xT": np.ascontiguousarray(xT), "xTq": xTq,
            "wqT": wqT, "wkS": wkS, "wvT": wvT,
            "cmask": cmasks[h],
        })
    return in_maps


def assemble(results):
    out = np.zeros((B, S, E), np.float32)
    for c in range(NCORES):
        b, h = c // 2, c % 2
        co = np.asarray(results[c]["out"], dtype=np.float32)
        for t in range(NQT):
            g = 2 * t + (1 - h)
            out[b, g * P:(g + 1) * P, :] = co[t * P:(t + 1) * P]
    return out


def kernel(x, W_q, W_k, W_v):
    from concourse.bass_utils import run_bass_kernel_spmd
    nc = build_program()
    in_maps = make_in_maps(x, W_q, W_k, W_v)
    res = run_bass_kernel_spmd(nc, in_maps, core_ids=list(range(NCORES)))
    return assemble(res.results)


# revision 59
# speedup vs baseline: 2.7163x; 1.0001x over previous
"""Collective-free causal attention: 8 cores = 4 batches x 2 q-stripe sets.

Core c = (batch b = c//2, stripe set h = c%2) owns the 8 query stripes
g = 2t + (1-h), t in 0..7, of batch b.  Each core projects the FULL K^T and V
for its batch locally (duplicated within the pair) instead of exchanging
halves over a collective — the cost model charges intra-pair AllGathers like
inter-chip transfers (15us + size/40GBps, serialized on one resource), which
dominated the original version; the duplicate K/V projection is ~27us of
extra PE work vs ~210us+ of modeled collective time.

Numerics: all matmul inputs are bf16 (host-converted); accumulation stays
fp32 in PSUM, softmax row sums stay fp32; outputs are stored bf16 and
widened to fp32 on the host.  Measured rel. Frobenius error ~5.4e-3.

Structure (single PE-dense stream, ~97% PE occupancy):
 - ~20 throwaway warm-up matmuls on a memset tile fill the input-DMA head
   so the cost model's PE clock ramp (half speed for the first 3us of a
   busy stretch) is spent before real work arrives.
 - Inputs land as a few large 3D-AP transfers (batched across the 8
   e-tiles), ordered so the K projection's first PSUM group is runnable
   ~5us in; W_k is host-shuffled to [k-chunk, p, e, c] so each k-chunk DMA
   moves 2KB rows and lands just ahead of the PE k-loop consuming it.
 - Projections K -> Q -> V (PSUM evictions alternate DVE/ACT); the
   single-block stripes t=0,1 compute scores/exp between Q and V and hold
   P^T + 1/rowsum in SBUF until the end.
 - Attention is stripe-major: each stripe walks its causal key blocks
   back-to-back, accumulating PV in one PSUM accumulation group (two
   stripes in flight across the vp pool's 2 tags x 3 bufs).  P^T is
   produced by an async SBUF->SBUF DMA transpose (XBAR) issued right after
   the exp, three items ahead of its PV, so its ~3us flight hides behind
   the scores stream — the PE never transposes.
 - Every stripe finalizes straight out of PSUM: the two feature halves are
   scaled by 1/rowsum on ACT and DVE in parallel (engine roles alternate
   per stripe) into a bf16 staging tile and stored with a single DMA.  The
   kernel tail is just PV -> scale -> store of a held 128-row stripe.
"""

import numpy as np

B, S, E, KD = 4, 2048, 1024, 1024
NCORES = 8
P = 128
ET = E // P          # 8 e-tiles of the contraction dim
KT = KD // P         # 8 k-tiles of Q^T/K^T partition dim
NQT = 8              # 8 q stripes of 128 per core
NBLK = 4             # 4 key blocks of 512
NST = S // P         # 16 key subtiles of 128 (V tiles)
NEG = -30000.0
SCALE = 1.0 / float(np.sqrt(KD))

_prog_cache = {}


def _n_blocks(t):
    return (t + 2) // 2


def _build_body(ctx, tc, ap):
    from concourse import mybir
    from concourse.masks import make_identity

    nc = tc.nc
    f32 = mybir.dt.float32
    bf16 = mybir.dt.bfloat16
    Exp = mybir.ActivationFunctionType.Exp
    X = mybir.AxisListType.X

    # batched [partition, e, cols] views of the inputs
    xTb = ap["xT"].rearrange("(e p) s -> p e s", p=P)      # [128, 8, 2048]
    xqb = ap["xTq"].rearrange("(e p) q -> p e q", p=P)     # [128, 8, 1024]
    wqb = ap["wqT"].rearrange("(e p) k -> p e k", p=P)
    # W_k is host-shuffled to [k-chunk, p, e, c] so each k-chunk DMA moves
    # 2KB-contiguous rows (256B rows would pay the sub-512B descriptor
    # penalty on the head-critical path)
    wkb = ap["wkS"].rearrange("k p e c -> k p (e c)")      # [8, 128, 1024]
    wvb = ap["wvT"].rearrange("(e p) f -> p e f", p=P)
    out_t = ap["out"].rearrange("(t p) f -> t p f", p=P)

    # ---- persistent tiles
    qt_pool = ctx.enter_context(tc.tile_pool(name="qt", bufs=1))
    QT = [qt_pool.tile([P, 1024], bf16, name=f"qt{k}", tag=f"qt{k}") for k in range(KT)]
    kt_pool = ctx.enter_context(tc.tile_pool(name="ktp", bufs=1))
    KTT = [kt_pool.tile([P, S], bf16, name=f"ktt{k}", tag=f"ktt{k}") for k in range(KT)]
    vv_pool = ctx.enter_context(tc.tile_pool(name="vvp", bufs=1))
    VV = [vv_pool.tile([P, E], bf16, name=f"vv{s}", tag=f"vv{s}") for s in range(NST)]
    rs_pool = ctx.enter_context(tc.tile_pool(name="rsp", bufs=1))
    RS = [rs_pool.tile([P, NBLK], f32, name=f"rs{t}", tag=f"rs{t}") for t in range(NQT)]
    const_pool = ctx.enter_context(tc.tile_pool(name="const", bufs=1))
    fin_pool = ctx.enter_context(tc.tile_pool(name="fin", bufs=4))

    # PSUM plan: sp lives for the whole kernel; pp (projection evictions,
    # 2 banks) is scoped to the projection phase and its banks are reused
    # by the attention vp pool (2 tags x 3 bufs = 6 banks; the handoff
    # dependency lands on the first PV matmuls, long after the last
    # projection eviction — no stall).
    sp = ctx.enter_context(tc.tile_pool(name="sp", bufs=2, space="PSUM"))

    # PE warm-up: the cost model runs the PE at 1/3.7 speed for the first
    # ~100ns of a busy stretch and at half speed until 3us of continuous
    # activity.  Fill the input-DMA head (~7us) with throwaway matmuls on a
    # memset tile so every real matmul runs at full rate.
    warm_sb = const_pool.tile([P, 256], bf16, name="warm_sb")
    nc.gpsimd.memset(warm_sb, 0)
    for i in range(22):
        wps = sp.tile([P, 256], f32, name="wps", tag="sp")
        nc.tensor.matmul(wps, warm_sb[:, :P], warm_sb, start=True, stop=True)

    p_pool = ctx.enter_context(tc.tile_pool(name="ppb", bufs=4))

    # GPSIMD cannot access PSUM, so evictions alternate DVE/ACT.
    evict_ops = [lambda d, s: nc.vector.tensor_copy(d, s),
                 lambda d, s: nc.scalar.copy(d, s)]
    evict_i = 0

    def evict(dst, src):
        nonlocal evict_i
        evict_ops[evict_i % 2](dst, src)
        evict_i += 1

    # ---- projection phase (scoped input pools + scoped eviction PSUM pool)
    held = {}  # t -> (P^T tiles, 1/rowsum) for the split stripes t=0,1
    with tc.tile_pool(name="xtp", bufs=1) as xt_pool, \
         tc.tile_pool(name="xqp", bufs=1) as xq_pool, \
         tc.tile_pool(name="wqp", bufs=1) as wq_pool, \
         tc.tile_pool(name="wkp", bufs=1) as wk_pool, \
         tc.tile_pool(name="wvp", bufs=1) as wv_pool, \
         tc.tile_pool(name="pp", bufs=2, space="PSUM") as pp:
        # x^T per key block sb: [p, (e 512)]  (slice e: [:, e*512:(e+1)*512])
        xts = [xt_pool.tile([P, ET * 512], bf16, name=f"xts{sb}", tag=f"xts{sb}")
               for sb in range(NBLK)]
        # W_k^T per k-chunk: [p, (e 128)]
        wkc = [wk_pool.tile([P, ET * P], bf16, name=f"wkc{k}", tag=f"wkc{k}")
               for k in range(KT)]
        wq_all = wq_pool.tile([P, ET * KD], bf16, name="wq", tag="wq")
        xq_all = xq_pool.tile([P, ET * 1024], bf16, name="xq", tag="xq")
        wv_all = wv_pool.tile([P, ET * E], bf16, name="wv", tag="wv")

        # DMA order tuned so the PE's first PSUM group is runnable ~5us in
        # and later chunks land just ahead of consumption.
        # first key block of x^T lands in two halves so the K projection's
        # e-accumulation can start after ~half the transfer
        xts0v = xts[0].rearrange("p (e s) -> p e s", s=512)
        nc.sync.dma_start(out=xts0v[:, 0:4, :], in_=xTb[:, 0:4, 0:512])
        nc.sync.dma_start(out=wkc[0], in_=wkb[0])
        nc.sync.dma_start(out=wkc[1], in_=wkb[1])
        nc.sync.dma_start(out=xts0v[:, 4:6, :], in_=xTb[:, 4:6, 0:512])
        nc.sync.dma_start(out=xts0v[:, 6:8, :], in_=xTb[:, 6:8, 0:512])
        for k in range(2, KT):
            nc.sync.dma_start(out=wkc[k], in_=wkb[k])
        for sb in range(1, NBLK):
            nc.sync.dma_start(
                out=xts[sb].rearrange("p (e s) -> p e s", s=512),
                in_=xTb[:, :, sb * 512:(sb + 1) * 512])
        nc.sync.dma_start(
            out=wq_all.rearrange("p (e k) -> p e k", k=KD), in_=wqb)
        nc.sync.dma_start(
            out=xq_all.rearrange("p (e q) -> p e q", q=1024), in_=xqb)
        nc.sync.dma_start(
            out=wv_all.rearrange("p (e f) -> p e f", f=E), in_=wvb)
        cm = const_pool.tile([P, 256], f32, name="cm")
        nc.sync.dma_start(out=cm, in_=ap["cmask"])

        # K^T[k] = W_k^T[:,k]^T x^T : [128 kdim, 2048 keys]
        for sb in range(NBLK):
            for k in range(KT):
                ps = pp.tile([P, 512], f32, name="ps_k", tag="pp")
                for e in range(ET):
                    nc.tensor.matmul(ps, wkc[k][:, e * P:(e + 1) * P],
                                     xts[sb][:, e * 512:(e + 1) * 512],
                                     start=(e == 0), stop=(e == ET - 1))
                evict(KTT[k][:, sb * 512:(sb + 1) * 512], ps)
        # Q^T[k] = W_q^T[:,k]^T xq^T : [128 kdim, 1024 own q rows]
        for qb in range(2):
            for k in range(KT):
                ps = pp.tile([P, 512], f32, name="ps_q", tag="pp")
                for e in range(ET):
                    nc.tensor.matmul(
                        ps, wq_all[:, e * KD + k * P: e * KD + (k + 1) * P],
                        xq_all[:, e * 1024 + qb * 512: e * 1024 + (qb + 1) * 512],
                        start=(e == 0), stop=(e == ET - 1))
                evict(QT[k][:, qb * 512:(qb + 1) * 512], ps)

        # scores/exp/transpose for the single-block stripes t=0,1 run here,
        # between the Q and V projections (their inputs — Q^T and key block
        # 0 — are ready); P^T and 1/rowsum are held in SBUF and only their
        # PV + store run at the very end of the kernel.
        for t in (1, 0):
            w = 256 * (t + 1)
            sps = sp.tile([P, 512], f32, name="sps", tag="sp")
            for k in range(KT):
                nc.tensor.matmul(sps[:, :w], QT[k][:, t * P:(t + 1) * P],
                                 KTT[k][:, :w], start=(k == 0),
                                 stop=(k == KT - 1))
            nc.vector.tensor_add(sps[:, w - 256:w], sps[:, w - 256:w], cm)
            pb = p_pool.tile([P, 512], bf16, name="pb", tag="pb")
            nc.scalar.activation(pb[:, :w], sps[:, :w], Exp, scale=SCALE,
                                 accum_out=RS[t][:, 0:1])
            ptile = fin_pool.tile([P, w], bf16, name="hpt", tag=f"hpt{t}",
                                  bufs=1)
            nc.sync.dma_start_transpose(
                out=ptile.rearrange("p (st c) -> p st c", c=P),
                in_=pb[:, :w])
            rinv = fin_pool.tile([P, 1], f32, name="hri", tag=f"hri{t}",
                                 bufs=1)
            nc.vector.reciprocal(rinv, RS[t][:, 0:1])
            held[t] = (ptile, w // P, rinv)

        # V[st] = x[st rows] W_v^T : [128 keys, 1024 features]
        for st in range(NST):
            sb, stv = st // 4, st % 4
            for fb in range(2):
                ps = pp.tile([P, 512], f32, name="ps_v", tag="pp")
                for e in range(ET):
                    nc.tensor.matmul(
                        ps, xts[sb][:, e * 512 + stv * P: e * 512 + (stv + 1) * P],
                        wv_all[:, e * E + fb * 512: e * E + (fb + 1) * 512],
                        start=(e == 0), stop=(e == ET - 1))
                evict(VV[st][:, fb * 512:(fb + 1) * 512], ps)

    # ---- attention phase
    vp = ctx.enter_context(tc.tile_pool(name="vp", bufs=2, space="PSUM"))
    pt_pool = ctx.enter_context(tc.tile_pool(name="ptp", bufs=5))

    # Attention is stripe-major: all of K^T/V is resident in SBUF, so each
    # stripe walks its key blocks back-to-back and accumulates PV entirely
    # in PSUM (vps holds both feature halves, one PSUM accumulation group
    # spanning the stripe's blocks).  No SBUF output accumulator, no DVE
    # adds, and every stripe finishes straight out of PSUM.  Only two
    # stripes' PV groups are ever in flight (vp tags x bufs=2 = 4 banks).
    cur_vps = {}
    fin_parity = [0]

    def scale_out(t, vps, rinv):
        obf = fin_pool.tile([P, E], bf16, name="obf", tag="obf", bufs=4)
        # scale the two halves on different engines in parallel (alternating
        # the assignment between consecutive finalizes so back-to-back
        # stripe finishes don't queue on one engine), then store with a
        # single DMA (HWDGE overhead is per-DMA)
        halves = [(0, vps[0]), (1, vps[1])]
        if fin_parity[0]:
            halves.reverse()
        fin_parity[0] ^= 1
        for i, (fb, src) in enumerate(halves):
            dst = obf[:, fb * 512:(fb + 1) * 512]
            if i == 0:
                nc.scalar.activation(dst, src,
                                     mybir.ActivationFunctionType.Copy,
                                     scale=rinv)
            else:
                nc.vector.tensor_scalar_mul(dst, src, rinv)
        nc.sync.dma_start(out=out_t[t], in_=obf)

    def finalize(t, vps):
        rsum = fin_pool.tile([P, 1], f32, name="rsum", tag="rsum")
        nc.vector.reduce_sum(rsum, RS[t][:, :_n_blocks(t)], axis=X)
        rinv = fin_pool.tile([P, 1], f32, name="rinv", tag="rinv")
        nc.vector.reciprocal(rinv, rsum)
        scale_out(t, vps, rinv)

    def emit_pv(pend):
        # deferred PV for one (t, blk) work item; P^T arrives via an async
        # DMA transpose issued right after the exp, two positions ahead, so
        # its ~3us flight time hides behind the scores stream.
        ptile, w, blk, t = pend
        nst = w // P
        if blk == 0:
            cur_vps[t] = [vp.tile([P, 512], f32, name=f"vps{fb}",
                                  tag=f"vp{fb}") for fb in range(2)]
        vps = cur_vps[t]
        is_final = (blk == _n_blocks(t) - 1)
        for st in range(nst):
            for fb in range(2):
                nc.tensor.matmul(vps[fb], ptile[:, st * P:(st + 1) * P],
                                 VV[4 * blk + st][:, fb * 512:(fb + 1) * 512],
                                 start=(blk == 0 and st == 0),
                                 stop=is_final and (st == nst - 1))
        if is_final:
            finalize(t, vps)

    # stripe-major schedule: big stripes first; the held single-block
    # stripes t=1,0 (scores pre-computed during the projection phase) come
    # last, so the kernel tail is just PV -> scale -> store.
    from collections import deque
    pend_q = deque()
    for t in (7, 3, 6, 2, 5, 4):
        for blk in range(_n_blocks(t)):
            w = min(512, 256 * (t + 1) - 512 * blk)
            is_diag = (blk == _n_blocks(t) - 1)
            sps = sp.tile([P, 512], f32, name="sps", tag="sp")
            for k in range(KT):
                nc.tensor.matmul(sps[:, :w], QT[k][:, t * P:(t + 1) * P],
                                 KTT[k][:, blk * 512: blk * 512 + w],
                                 start=(k == 0), stop=(k == KT - 1))
            if is_diag:
                nc.vector.tensor_add(sps[:, w - 256:w], sps[:, w - 256:w], cm)
            pb = p_pool.tile([P, 512], bf16, name="pb", tag="pb")
            nc.scalar.activation(pb[:, :w], sps[:, :w], Exp, scale=SCALE,
                                 accum_out=RS[t][:, blk:blk + 1])
            ptile = pt_pool.tile([P, 512], bf16, name="ptd", tag="ptd")
            nc.sync.dma_start_transpose(
                out=ptile.rearrange("p (st c) -> p st c", c=P)[:, :w // P, :],
                in_=pb[:, :w])
            pend_q.append((ptile, w, blk, t))
            if len(pend_q) > 3:
                emit_pv(pend_q.popleft())

    def emit_held_pv(t):
        # PV + PSUM-direct scale + store from pre-computed P^T and 1/rowsum
        ptile, nst, rinv = held[t]
        vps = [vp.tile([P, 512], f32, name=f"vps{fb}", tag=f"vp{fb}")
               for fb in range(2)]
        for st in range(nst):
            for fb in range(2):
                nc.tensor.matmul(vps[fb], ptile[:, st * P:(st + 1) * P],
                                 VV[st][:, fb * 512:(fb + 1) * 512],
                                 start=(st == 0), stop=(st == nst - 1))
        scale_out(t, vps, rinv)

    # drain: interleave the held stripes' PVs so each store chain hides
    # under the next stripe's PE work and the last in-flight DMA transpose
    # gets PE work to hide behind; only the very last store is exposed.
    emit_pv(pend_q.popleft())
    emit_pv(pend_q.popleft())
    emit_held_pv(1)
    emit_pv(pend_q.popleft())
    emit_held_pv(0)


def build_program():
    if "nc" in _prog_cache:
        return _prog_cache["nc"]
    from contextlib import ExitStack
    from concourse import bacc, mybir
    import concourse.tile as tile

    nc = bacc.Bacc("TRN2", target_bir_lowering=False, debug=False,
                   num_devices=NCORES)
    f32 = mybir.dt.float32
    bf16 = mybir.dt.bfloat16
    ap = {
        "xT": nc.dram_tensor("xT", [E, S], bf16, kind="ExternalInput").ap(),
        "xTq": nc.dram_tensor("xTq", [E, 1024], bf16, kind="ExternalInput").ap(),
        "wqT": nc.dram_tensor("wqT", [E, KD], bf16, kind="ExternalInput").ap(),
        "wkS": nc.dram_tensor("wkS", [KT, P, ET, P], bf16,
                              kind="ExternalInput").ap(),
        "wvT": nc.dram_tensor("wvT", [E, E], bf16, kind="ExternalInput").ap(),
        "cmask": nc.dram_tensor("cmask", [P, 256], f32, kind="ExternalInput").ap(),
        "out": nc.dram_tensor("out", [1024, E], bf16, kind="ExternalOutput").ap(),
    }
    with tile.TileContext(nc) as tc:
        with ExitStack() as ctx:
            _build_body(ctx, tc, ap)
    nc.compile()
    _prog_cache["nc"] = nc
    return nc


def make_in_maps(x, W_q, W_k, W_v):
    import ml_dtypes
    bf16 = ml_dtypes.bfloat16
    x = np.asarray(x, np.float32)
    wqT = np.ascontiguousarray(np.asarray(W_q, np.float32).T.astype(bf16))
    wkT = np.asarray(W_k, np.float32).T.astype(bf16)
    # [k-chunk, p, e, c]: wkS[k, p, e, c] = wkT[e*128+p, k*128+c]
    wkS = np.ascontiguousarray(
        wkT.reshape(ET, P, KT, P).transpose(2, 1, 0, 3))
    wvT = np.ascontiguousarray(np.asarray(W_v, np.float32).T.astype(bf16))
    i = np.arange(P)[:, None]
    j = np.arange(256)[None, :]
    cmasks = [np.where(j <= i + 128, 0.0, NEG).astype(np.float32),
              np.where(j <= i, 0.0, NEG).astype(np.float32)]
    in_maps = []
    for c in range(NCORES):
        b, h = c // 2, c % 2
        xT = x[b].T.astype(bf16)
        qtiles = [2 * t + (1 - h) for t in range(NQT)]
        qcols = np.concatenate([np.arange(g * P, (g + 1) * P) for g in qtiles])
        xTq = np.ascontiguousarray(xT[:, qcols])
        in_maps.append({
            "xT": np.ascontiguousarray(xT), "xTq": xTq,
            "wqT": wqT, "wkS": wkS, "wvT": wvT,
            "cmask": cmasks[h],
        })
    return in_maps


def assemble(results):
    out = np.zeros((B, S, E), np.float32)
    for c in range(NCORES):
        b, h = c // 2, c % 2
        co = np.asarray(results[c]["out"], dtype=np.float32)
        for t in range(NQT):
            g = 2 * t + (1 - h)
            out[b, g * P:(g + 1) * P, :] = co[t * P:(t + 1) * P]
    return out


def kernel(x, W_q, W_k, W_v):
    from concourse.bass_utils import run_bass_kernel_spmd
    nc = build_program()
    in_maps = make_in_maps(x, W_q, W_k, W_v)
    res = run_bass_kernel_spmd(nc, in_maps, core_ids=list(range(NCORES)))
    return assemble(res.results)
